# revision 53
# baseline (speedup 1.0000x reference)
"""Dual-path multi-head attention on 8 trn2 NeuronCores.

Sharding: core c = (path p=c//4, batch b=c%4). Each core runs the full
pipeline for one path and one batch element: 3 input projections, 16-head
attention (S=1024, dh=64), output projection. No collectives.

Path 2 cross-wiring (q2 from k; k2,v2 from q) is handled purely by host-side
input routing - every core runs the identical SPMD program.

Device layouts (per core, all pre-packed on host for contiguous DMA runs):
  xq/xk/xv : [p, n, s]   = x.T blocked:  x[s, n*128+p]
  wq/wc    : [p, m, n, e'] = W[m*128+e', n*128+p]  (W.T blocked by out-block m)
  wv       : [p, n, e]   = Wv[e, n*128+p]
  Projections compute Q1T/K1T = [e, s] and V1 = [s, e]; scores are computed
  transposed (probs_T[sk, sq]) so softmax needs no transposes. Softmax is
  max-free (scores ~ N(0,1)); the denominator comes from a ones-column
  appended per head slot in v1e.

PV runs "flipped": out[sq-block, dh+1] with probs_T blocks as the stationary
operand, so all 128 output partitions are used (half the PE rows of the
[dh+1, sq] orientation). The per-(sq,head) denominators land on the same
partition as their row, so normalization is a per-partition reciprocal +
broadcast multiply - no cross-partition broadcast needed. The normalized
attention output a[sq, d] is then transposed back to [d, sq] with PE
transpose instructions (identity permutation) to feed the output projection.

Scheduling: the exp chain keeps the Activation engine ~95% busy inside the
head loop, so every other PE-side task is emitted as filler between the
exp-gated score/PV items: the next pair's Q/K projections, the previous
pair's transposes, V-proj block 7, and (in the last pair) the first output
projection block. Each head's PV tail + normalize is deferred into the next
head's early chunks so the next score chunk isn't emission-blocked behind
exp-gated PV work.
"""

import collections

import numpy as np
import ml_dtypes

B, S, D, H, DH = 4, 1024, 1024, 16, 64
NB = D // 128  # 8 partition-blocks
HW = 65  # head slot width in v1e (64 data + 1 ones col)

_compiled = None


def _build():
    import concourse.bass as bass
    import concourse.mybir as mybir
    import concourse.tile as tile
    from concourse import bacc

    dt = mybir.dt
    f32, bf16, f32r = dt.float32, dt.bfloat16, dt.float32r

    nc = bacc.Bacc("TRN2", target_bir_lowering=False, debug=False)

    xq_d = nc.dram_tensor("xq", [128, NB, S], bf16, kind="ExternalInput")
    xk_d = nc.dram_tensor("xk", [128, NB, S], bf16, kind="ExternalInput")
    xv_d = nc.dram_tensor("xv", [128, NB, S], bf16, kind="ExternalInput")
    wq_d = nc.dram_tensor("wq", [128, NB, NB, 128], bf16, kind="ExternalInput")
    wk_d = nc.dram_tensor("wk", [128, NB, NB, 128], bf16, kind="ExternalInput")
    wv_d = nc.dram_tensor("wv", [128, NB, D], bf16, kind="ExternalInput")
    wc_d = nc.dram_tensor("wc", [128, NB, NB, 128], bf16, kind="ExternalInput")
    bq_d = nc.dram_tensor("bq", [128, NB], f32, kind="ExternalInput")
    bk_d = nc.dram_tensor("bk", [128, NB], f32, kind="ExternalInput")
    bc_d = nc.dram_tensor("bc", [128, NB], f32, kind="ExternalInput")
    bvB_d = nc.dram_tensor("bvB", [128, D], bf16, kind="ExternalInput")
    id_d = nc.dram_tensor("ident", [128, 128], bf16, kind="ExternalInput")
    out_d = nc.dram_tensor("outT", [D, S], bf16, kind="ExternalOutput")

    ExpF = mybir.ActivationFunctionType.Exp

    with tile.TileContext(nc) as tc:
        with tc.tile_pool(name="x", bufs=3) as xp, \
             tc.tile_pool(name="wfull", bufs=1) as wfp, \
             tc.tile_pool(name="wblk", bufs=6) as wbp, \
             tc.tile_pool(name="cst", bufs=1) as cp, \
             tc.tile_pool(name="qk", bufs=4) as qkp, \
             tc.tile_pool(name="pers", bufs=1) as prp, \
             tc.tile_pool(name="pt", bufs=2) as ptp, \
             tc.tile_pool(name="rcp", bufs=2) as rcp, \
             tc.tile_pool(name="ost", bufs=2) as ostp, \
             tc.tile_pool(name="vp", bufs=2, space="PSUM") as vpp, \
             tc.tile_pool(name="mm", bufs=2, space="PSUM") as mmp, \
             tc.tile_pool(name="pv", bufs=1, space="PSUM") as pvp:

            # ---- warmups: a memset-fed dummy matmul starts the PE p-state
            # ramp clock (~full rate by the time V-proj is fed), and a tiny
            # exp pulls LoadActFuncSet (~1.3us) off the critical path.
            wmm = cp.tile([1, 2], bf16, name="wmm")
            nc.vector.memset(wmm[:, :], 0.5)
            wps = vpp.tile([1, 2], f32, tag="vp", name="wps")
            nc.tensor.matmul(wps[:, :], wmm[:, 0:1], wmm[:, :], start=True, stop=True)
            warm = cp.tile([1, 2], bf16, name="warm")
            nc.scalar.activation(out=warm[:, :], in_=wmm[:, :], func=ExpF)

            # ---- loads: V-proj feed first (xv/wv blocks interleaved, first
            # blocks split for an earlier first matmul), tiny biases after
            # the big x tensors so they don't delay the feed.
            xv_t = xp.tile([128, NB, S], bf16, tag="x")
            wv_t = wfp.tile([128, NB, D], bf16)
            nc.sync.dma_start(out=xv_t[:, 0, 0:512], in_=xv_d.ap()[:, 0, 0:512])
            nc.sync.dma_start(out=wv_t[:, 0, 0:512], in_=wv_d.ap()[:, 0, 0:512])
            nc.sync.dma_start(out=wv_t[:, 0, 512:D], in_=wv_d.ap()[:, 0, 512:D])
            nc.sync.dma_start(out=xv_t[:, 0, 512:S], in_=xv_d.ap()[:, 0, 512:S])
            nc.sync.dma_start(out=xv_t[:, 1, :], in_=xv_d.ap()[:, 1, :])
            nc.sync.dma_start(out=wv_t[:, 1, :], in_=wv_d.ap()[:, 1, :])
            bvB_t = cp.tile([128, D], bf16)
            nc.sync.dma_start(out=bvB_t[:, :], in_=bvB_d.ap())
            for n in range(2, NB):
                nc.sync.dma_start(out=xv_t[:, n, :], in_=xv_d.ap()[:, n, :])
                nc.sync.dma_start(out=wv_t[:, n, :], in_=wv_d.ap()[:, n, :])
            # small weight blocks ahead of the big x tensors: proj0 follows
            # right behind the xq/xk stream instead of waiting on blocks
            # queued after it
            wqb = wbp.tile([128, NB, 128], bf16, tag="wblk", name="wq0b")
            nc.sync.dma_start(out=wqb[:, :, :], in_=wq_d.ap()[:, 0, :, :])
            wkb = wbp.tile([128, NB, 128], bf16, tag="wblk", name="wk0b")
            nc.sync.dma_start(out=wkb[:, :, :], in_=wk_d.ap()[:, 0, :, :])
            xq_t = xp.tile([128, NB, S], bf16, tag="x")
            nc.sync.dma_start(out=xq_t[:, :, :], in_=xq_d.ap())
            xk_t = xp.tile([128, NB, S], bf16, tag="x")
            nc.sync.dma_start(out=xk_t[:, :, :], in_=xk_d.ap())
            bq_t = cp.tile([128, NB], f32)
            nc.sync.dma_start(out=bq_t[:, :], in_=bq_d.ap())
            bk_t = cp.tile([128, NB], f32)
            nc.sync.dma_start(out=bk_t[:, :], in_=bk_d.ap())
            bc_t = cp.tile([128, NB], f32)
            nc.sync.dma_start(out=bc_t[:, :], in_=bc_d.ap())
            id_t = cp.tile([128, 128], bf16)
            nc.sync.dma_start(out=id_t[:, :], in_=id_d.ap())

            v1e = prp.tile([128, NB, H * HW], bf16)
            # a[sq, d] per sq-block, written head by head, bf16
            asq = xp.tile([128, NB, H, DH], bf16, tag="x", name="asq")
            # a^T blocks [d-chunk, s] feeding the output projection
            aT = [prp.tile([128, S], bf16, tag=f"aT_{n}", name=f"aT_{n}")
                  for n in range(NB)]

            # ones columns of v1e (softmax denominator trick)
            ones_ap = v1e[:, :, :].rearrange("p n (h x) -> p n h x", x=HW)[:, :, :, 64]
            nc.vector.memset(ones_ap, 1.0)

            def vproj_block(n2):
                ps = mmp.tile([128, 2, 512], f32, tag="mm", name=f"vps{n2}")
                for n in range(NB):
                    for c in range(2):
                        nc.tensor.matmul(
                            ps[:, c, :],
                            xv_t[:, n, n2 * 128:(n2 + 1) * 128],
                            wv_t[:, n, c * 512:(c + 1) * 512],
                            start=(n == 0), stop=(n == NB - 1),
                        )
                dst = v1e[:, n2, :].rearrange("p (c h x) -> p c h x", c=2, x=HW)[:, :, :, 0:64]
                ps_v = ps[:, :, :].rearrange("p c (h x) -> p c h x", x=64)
                bv_v = bvB_t[:, :].rearrange("p (c h x) -> p c h x", c=2, x=64)
                nc.vector.tensor_add(dst, ps_v, bv_v)

            def make_vproj_work(n2):
                """V-proj block as 16 filler thunks (two vp-ring half chains)."""
                st = {}
                thunks = []
                for c in range(2):
                    for n in range(NB):
                        def stp(c=c, n=n):
                            if n == 0:
                                st[c] = vpp.tile([128, 512], f32, tag="vp",
                                                 name=f"vpw{n2}_{c}")
                            nc.tensor.matmul(
                                st[c][:, :],
                                xv_t[:, n, n2 * 128:(n2 + 1) * 128],
                                wv_t[:, n, c * 512:(c + 1) * 512],
                                start=(n == 0), stop=(n == NB - 1),
                            )
                            if n == NB - 1:
                                v1v = v1e[:, n2, :].rearrange(
                                    "p (c h x) -> p c h x", c=2, x=HW)
                                bv_v = bvB_t[:, :].rearrange(
                                    "p (c h x) -> p c h x", c=2, x=64)
                                nc.vector.tensor_add(
                                    v1v[:, c, :, 0:64],
                                    st[c][:, :].rearrange("p (h x) -> p h x", x=64),
                                    bv_v[:, c])
                        thunks.append(stp)
                return thunks

            def wblk_load(w_d, m):
                wb = wbp.tile([128, NB, 128], bf16, tag="wblk")
                nc.sync.dma_start(out=wb[:, :, :], in_=w_d.ap()[:, m, :, :])
                return wb

            def proj_block(wb, x_t, b_t, m):
                """[e-block m, s] = W.T-block @ x.T (+ bias) -> f32 tile.
                Kept in f32 so the scores matmuls can run in float32r
                (full-rate for moving dim >= 256) for better accuracy."""
                ob = qkp.tile([128, S], f32r, tag="qk")
                for c in range(2):
                    ps = vpp.tile([128, 512], f32, tag="vp", name=f"pjps{m}_{c}")
                    for n in range(NB):
                        nc.tensor.matmul(
                            ps[:, :], wb[:, n, :], x_t[:, n, c * 512:(c + 1) * 512],
                            start=(n == 0), stop=(n == NB - 1),
                        )
                    nc.vector.tensor_scalar_add(
                        ob[:, c * 512:(c + 1) * 512], ps[:, :], b_t[:, m:m + 1])
                return ob

            def make_proj_work(wb, x_t, b_t, m):
                """Per-matmul thunks for one projection, spread into the head
                stream as PE filler between exp-gated items."""
                ob = qkp.tile([128, S], f32r, tag="qk", name=f"ob{m}")
                st = {}
                thunks = []
                for c in range(2):
                    for k in range(NB):
                        def step(c=c, k=k):
                            if k == 0:
                                st[c] = vpp.tile([128, 512], f32, tag="vp",
                                                 name=f"cps{m}_{c}")
                            nc.tensor.matmul(
                                st[c][:, :], wb[:, k, :],
                                x_t[:, k, c * 512:(c + 1) * 512],
                                start=(k == 0), stop=(k == NB - 1),
                            )
                            if k == NB - 1:
                                nc.vector.tensor_scalar_add(
                                    ob[:, c * 512:(c + 1) * 512], st[c][:, :],
                                    b_t[:, m:m + 1])
                        thunks.append(step)
                return ob, thunks

            def make_transpose_work(mt):
                """Two thunks of 4 PE transposes each + DVE drain: asq pair
                mt -> aT[mt]."""
                def half(lo):
                    def f():
                        tp = mmp.tile([128, 4, 128], bf16, tag="mm",
                                      name=f"tp{mt}_{lo}")
                        for b in range(4):
                            nc.tensor.transpose(
                                tp[:, b, :],
                                asq[:, lo + b, 2 * mt:2 * mt + 2, :],
                                id_t[:, :])
                        nc.vector.tensor_copy(
                            aT[mt][:, :].rearrange(
                                "p (b j) -> p b j", j=128)[:, lo:lo + 4, :],
                            tp[:, :, :])
                    return f
                return [half(0), half(4)]

            def make_outproj_prefill():
                """Output-proj block m=0, contraction chunks n=0..6, as filler
                for the last pair (whose queue has no projection work). The
                two vp-ring chains stay open until finish_outproj_prefill."""
                st = {}

                def stp(c, n):
                    def f():
                        if n == 0:
                            st[c] = vpp.tile([128, 512], f32, tag="vp",
                                             name=f"opre{c}")
                        nc.tensor.matmul(
                            st[c][:, :], wc_t[:, 0, n, :],
                            aT[n][:, c * 512:(c + 1) * 512],
                            start=(n == 0), stop=False,
                        )
                    return f

                thunks = [stp(c, n) for c in range(2) for n in range(NB - 2)]
                return st, thunks, [stp(c, NB - 2) for c in range(2)]

            def pv_chunk(pvps, pt, h, n):
                # all start=False: a start would mark the whole 2KB PSUM
                # zero-region pending-zero and clobber the other sq-block
                # chains sharing the bank. pvps is DVE-memset to zero instead.
                for b in range(NB):
                    nc.tensor.matmul(
                        pvps[:, b, 0:HW],
                        pt[:, n, b * 128:(b + 1) * 128],
                        v1e[:, n, h * HW:(h + 1) * HW],
                        start=False, stop=(n == NB - 1),
                        skip_group_check=True,
                    )

            def norm_head(pvps, h):
                rec = rcp.tile([128, NB, 1], f32, tag="rec", name=f"rec{h}")
                nc.vector.reciprocal(rec[:, :, 0], pvps[:, :, 64])
                nc.vector.tensor_mul(
                    asq[:, :, h, :], pvps[:, :, 0:64],
                    rec[:, :, :].broadcast_to((128, NB, 64)))

            def head(h, q1b, k1b, work, prev_tail, slot0, tail_at=1):
                """Emit one head's score/exp stream with PE filler; PV tail +
                normalize are returned as a closure the caller defers into the
                next head. slot0 = this head's first slot index in the pair
                (for even filler spreading). tail_at delays the deferred tail
                (and this head's own PV start) to free early filler slots."""
                po = (h % 2) * 64
                pt = ptp.tile([128, NB, S], bf16, tag="pt", name=f"pt{h}")
                st = {"next_pv": 0}
                for n in range(NB):
                    # filler first: at head/pair boundaries the score matmul
                    # below waits on the exp chain catching up, so give the
                    # PE exp-independent work to chew through first
                    slots_left = 16 - (slot0 + n)
                    npop = min(len(work), max(2, -(-len(work) // max(slots_left, 1))))
                    for _ in range(npop):
                        work.popleft()()
                    sps = mmp.tile([128, 2, 512], f32, tag="mm", name=f"sps{h}_{n}")
                    for c in range(2):
                        nc.tensor.matmul(
                            sps[:, c, :],
                            k1b[po:po + 64, n * 128:(n + 1) * 128],
                            q1b[po:po + 64, c * 512:(c + 1) * 512],
                            start=True, stop=True,
                        )
                    nc.scalar.activation(
                        out=pt[:, n, :].rearrange("p (c s) -> p c s", c=2),
                        in_=sps[:, :, :], func=ExpF, scale=0.125)
                    if n == tail_at:
                        if prev_tail is not None:
                            prev_tail()
                        pvps = pvp.tile([128, NB, 128], f32, tag="pv",
                                        name=f"pv{h}")
                        nc.vector.memset(pvps[:, :, 0:HW], 0.0)
                        st["pv"] = pvps
                    if n > tail_at:
                        # up to 2 PV chunks per slot, catching up to lag-2
                        emitted = 0
                        while st["next_pv"] <= n - 5 and emitted < 2:
                            pv_chunk(st["pv"], pt, h, st["next_pv"])
                            st["next_pv"] += 1
                            emitted += 1

                def tail(fill=()):
                    fl = collections.deque(fill)
                    while st["next_pv"] <= NB - 2:
                        pv_chunk(st["pv"], pt, h, st["next_pv"])
                        st["next_pv"] += 1
                    for _ in range(min(3, len(fl))):
                        fl.popleft()()
                    pv_chunk(st["pv"], pt, h, NB - 1)
                    while fl:
                        fl.popleft()()
                    norm_head(st["pv"], h)
                return tail

            # ---- V projection. Blocks 0+1 run n-major with both mm-ring
            # chains open (free until scores start): the PE consumes each
            # xv/wv block-pair as it lands instead of stalling inside block
            # 0's contraction until the whole stream has arrived.
            ps01 = [mmp.tile([128, 2, 512], f32, tag="mm", name=f"vps0{i}")
                    for i in range(2)]
            for n in range(NB):
                for n2 in range(2):
                    for c in range(2):
                        nc.tensor.matmul(
                            ps01[n2][:, c, :],
                            xv_t[:, n, n2 * 128:(n2 + 1) * 128],
                            wv_t[:, n, c * 512:(c + 1) * 512],
                            start=(n == 0), stop=(n == NB - 1),
                        )
            bv_v = bvB_t[:, :].rearrange("p (c h x) -> p c h x", c=2, x=64)
            for n2 in range(2):
                dst = v1e[:, n2, :].rearrange(
                    "p (c h x) -> p c h x", c=2, x=HW)[:, :, :, 0:64]
                nc.vector.tensor_add(
                    dst, ps01[n2][:, :, :].rearrange("p c (h x) -> p c h x", x=64),
                    bv_v)
            for n2 in range(2, NB - 3):
                vproj_block(n2)
            q1b = proj_block(wqb, xq_t, bq_t, 0)
            vproj_block(NB - 3)
            k1b = proj_block(wkb, xk_t, bk_t, 0)
            vproj_block(NB - 2)

            # ---- head loop: pair m = heads (2m, 2m+1) ----
            # weight blocks prefetched two pairs ahead so the proj filler
            # popped at a pair's first chunks never waits on its DMA
            nxt = [wblk_load(wq_d, 1), wblk_load(wk_d, 1)]
            wc_t = None
            prefill = None
            prev_tail = None
            for m in range(NB):
                work = collections.deque()
                if m == 0:
                    work.extend(make_vproj_work(NB - 1))
                if m == 1:
                    wc_t = wfp.tile([128, NB, NB, 128], bf16, tag="wc")
                    nc.sync.dma_start(out=wc_t[:, :, :, :], in_=wc_d.ap())
                if m < NB - 1:
                    nwqb, nwkb = nxt
                    if m < NB - 2:
                        nxt = [wblk_load(wq_d, m + 2), wblk_load(wk_d, m + 2)]
                    nq1b, tq = make_proj_work(nwqb, xq_t, bq_t, m + 1)
                    nk1b, tk = make_proj_work(nwkb, xk_t, bk_t, m + 1)
                    work.extend(tq)
                    work.extend(tk)
                    if m >= 1:
                        work.extend(make_transpose_work(m - 1))
                else:
                    # prefill chunks n<=5 first (their aT blocks are ready),
                    # then pair-6 transposes (their asq needs tail(13), which
                    # is emitted at head-14 chunk 1), then the n=6 chunks that
                    # depend on those transposes
                    prefill, pfthunks, pftail = make_outproj_prefill()
                    work.extend(pfthunks)
                    work.extend(make_transpose_work(m - 1))
                    work.extend(pftail)
                prev_tail = head(2 * m, q1b, k1b, work, prev_tail, 0)
                # head 1 defers head-0's tail (PV of v-proj blocks 6/7) until
                # the pair-0 filler has produced v1e blocks 6/7
                prev_tail = head(2 * m + 1, q1b, k1b, work, prev_tail, 8,
                                 tail_at=2 if m == 0 else 1)
                while work:
                    work.popleft()()
                if m < NB - 1:
                    q1b, k1b = nq1b, nk1b

            # ---- loop/output-projection transition: head-15's PV tail +
            # normalize interleave with the m=1 out-proj accumulation so the
            # PE never waits on the exp tail or the DVE normalize.
            m1ps = mmp.tile([128, 2, 512], f32, tag="mm", name="ops1")

            def m1mm(n, last=False):
                def f():
                    for c in range(2):
                        nc.tensor.matmul(
                            m1ps[:, c, :], wc_t[:, 1, n, :],
                            aT[n][:, c * 512:(c + 1) * 512],
                            start=(n == 0), stop=last,
                        )
                return f

            prev_tail([m1mm(n) for n in range(5)])
            # last pair's transposes: one PSUM tile so the open m1 chain and
            # the transposes don't fight over the two mm-ring slots; the
            # drains run on the Activation engine (idle after the last exp),
            # split per half with the m1 chunks as PE filler under the copy
            # latency so block 0's finish doesn't wait the full copy
            tp7 = mmp.tile([128, NB, 128], bf16, tag="mm", name="tp7")
            aT7v = aT[NB - 1][:, :].rearrange("p (b j) -> p b j", j=128)
            for b in range(NB):
                nc.tensor.transpose(
                    tp7[:, b, :], asq[:, b, 2 * NB - 2:2 * NB, :], id_t[:, :])
            nc.scalar.copy(aT7v[:, 0:4, :], tp7[:, 0:4, :])
            m1mm(5)()
            nc.scalar.copy(aT7v[:, 4:8, :], tp7[:, 4:8, :])
            m1mm(6)()

            # finish block 0 from the prefilled vp chains
            for c in range(2):
                nc.tensor.matmul(
                    prefill[c][:, :], wc_t[:, 0, NB - 1, :],
                    aT[NB - 1][:, c * 512:(c + 1) * 512],
                    start=False, stop=True,
                )
                ot = ostp.tile([128, 512], bf16, tag="ostl")
                nc.scalar.add(ot[:, :], prefill[c][:, :], bc_t[:, 0:1])
                nc.sync.dma_start(
                    out=out_d.ap()[0:128, c * 512:(c + 1) * 512], in_=ot[:, :])
            # finish block 1
            m1mm(NB - 1, last=True)()
            ot1 = ostp.tile([128, 2, 512], bf16, tag="ost")
            nc.scalar.add(ot1[:, :, :], m1ps[:, :, :], bc_t[:, 1:2])
            nc.sync.dma_start(
                out=out_d.ap()[128:256, :].rearrange("p (c s) -> p c s", c=2),
                in_=ot1[:, :, :])

            for m in range(2, NB):
                ops = mmp.tile([128, 2, 512], f32, tag="mm", name=f"ops{m}")
                if m < NB - 1:
                    for n in range(NB):
                        for c in range(2):
                            nc.tensor.matmul(
                                ops[:, c, :], wc_t[:, m, n, :],
                                aT[n][:, c * 512:(c + 1) * 512],
                                start=(n == 0), stop=(n == NB - 1),
                            )
                    ot = ostp.tile([128, 2, 512], bf16, tag="ost")
                    # alternate drain engines so two block-drains can be in
                    # flight and the mm-ring WAR never waits a serialized one
                    if m % 2 == 0:
                        nc.scalar.add(ot[:, :, :], ops[:, :, :], bc_t[:, m:m + 1])
                    else:
                        nc.vector.tensor_scalar_add(ot[:, :, :], ops[:, :, :],
                                                    bc_t[:, m:m + 1])
                    nc.sync.dma_start(
                        out=out_d.ap()[m * 128:(m + 1) * 128, :].rearrange(
                            "p (c s) -> p c s", c=2),
                        in_=ot[:, :, :])
                else:
                    # last block: chunk-major so c0's drain+store hides under
                    # c1's accumulation
                    for c in range(2):
                        for n in range(NB):
                            nc.tensor.matmul(
                                ops[:, c, :], wc_t[:, m, n, :],
                                aT[n][:, c * 512:(c + 1) * 512],
                                start=(n == 0), stop=(n == NB - 1),
                            )
                        ot = ostp.tile([128, 512], bf16, tag="ostl")
                        nc.scalar.add(ot[:, :], ops[:, c, :], bc_t[:, m:m + 1])
                        nc.sync.dma_start(
                            out=out_d.ap()[m * 128:(m + 1) * 128,
                                           c * 512:(c + 1) * 512],
                            in_=ot[:, :])

    nc.compile()
    return nc


def _get_nc():
    global _compiled
    if _compiled is None:
        _compiled = _build()
    return _compiled


def _make_in_maps(q, k, v, Wq, bq, Wk, bk, Wv, bv, Wq2, bq2, Wk2, bk2, Wv2, bv2,
                  Wc, bc, Wc2, bc2):
    bf16 = ml_dtypes.bfloat16

    def xpack(x):  # [s, d] -> [p, n, s]
        x = np.asarray(x, np.float32)
        return np.ascontiguousarray(x.reshape(S, NB, 128).transpose(2, 1, 0)).astype(bf16)

    def wpack(w):  # W[e, d] -> [p, m, n, e']
        w = np.asarray(w, np.float32)
        return np.ascontiguousarray(
            w.reshape(NB, 128, NB, 128).transpose(3, 0, 2, 1)).astype(bf16)

    def wvpack(w):  # Wv[e, d] -> [p, n, e]
        w = np.asarray(w, np.float32)
        return np.ascontiguousarray(w.T.reshape(NB, 128, D).transpose(1, 0, 2)).astype(bf16)

    def btile(b):
        return np.ascontiguousarray(np.asarray(b, np.float32).reshape(NB, 128).T)

    def brep(b):
        return np.ascontiguousarray(
            np.broadcast_to(np.asarray(b, np.float32), (128, D))).astype(bf16)

    ident = np.eye(128, dtype=bf16)

    paths = [
        dict(wq=wpack(Wq), wk=wpack(Wk), wv=wvpack(Wv), wc=wpack(Wc),
             bq=btile(bq), bk=btile(bk), bc=btile(bc), bvB=brep(bv), ident=ident),
        dict(wq=wpack(Wq2), wk=wpack(Wk2), wv=wvpack(Wv2), wc=wpack(Wc2),
             bq=btile(bq2), bk=btile(bk2), bc=btile(bc2), bvB=brep(bv2), ident=ident),
    ]
    in_maps = []
    for c in range(8):
        p, b = c // 4, c % 4
        if p == 0:
            xq, xk, xv = xpack(q[b]), xpack(k[b]), xpack(v[b])
        else:
            # path 2: q2 from k; k2, v2 from q
            xq, xk, xv = xpack(k[b]), xpack(q[b]), xpack(q[b])
        in_maps.append(dict(paths[p], xq=xq, xk=xk, xv=xv))
    return in_maps


def _run(in_maps, trace=False):
    from concourse.bass_utils import run_bass_kernel_spmd
    nc = _get_nc()
    return run_bass_kernel_spmd(nc, in_maps, core_ids=list(range(8)), trace=trace)


def kernel(**inputs):
    in_maps = _make_in_maps(**inputs)
    try:
        res = _run(in_maps)
    except Exception:
        # transient NRT_EXEC_UNIT_UNRECOVERABLE has been observed when a
        # prior process crashed mid-execution; one retry reloads the NEFF
        res = _run(in_maps)
    out1 = np.stack([res.results[b]["outT"].astype(np.float32).T
                     for b in range(4)])
    out2 = np.stack([res.results[4 + b]["outT"].astype(np.float32).T
                     for b in range(4)])
    return out1, out2
